# revision 1
# baseline (speedup 1.0000x reference)
"""MHA forward kernel for Trainium2 (Bass/Tile), sharded over (batch, head)
pairs across 8 NeuronCores.

Math (per (b,h) pair):
    scores = softmax(Q K^T / sqrt(64) + bias)   # bias broadcast over (b,h)
    out    = scores @ V

Device-side layout: everything is computed TRANSPOSED so the big S x S
scores matrix never needs an on-chip transpose:
    scoresT[k, q] = sum_d K[k,d] Q'[q,d]       (Q' = Q/8, pre-scaled once)
    p = exp(scoresT + biasT)                   (no max-subtraction: scores
                                                ~ N(0,2), exp safe in f32)
    outT[d, q], sums[q] = [V | ones] matmul accumulating over k
    out[q, d] = outT[d, q] / sums[q]           (PE transpose + per-row recip)

Engine balancing: matmuls run as float32r (1 cyc/row vs 4 for fp32); all
matmul operands live in f32r-typed tiles (BIR requires f32r-rounded
producers). The bias add is split: PE-path groups get bias added via an
identity-matmul accumulate into PSUM; DVE-path groups use
scalar_tensor_tensor. exp always runs on ACT (PSUM source for PE-path,
SBUF in-place for DVE-path). MM2 consumption is software-pipelined one
chunk behind production, epilogues two chunks behind, so the PE never
waits on exp.
"""

import os
import sys

import numpy as np

for _p in ("/opt/trn_rl_repo",):
    if _p not in sys.path and os.path.isdir(_p):
        sys.path.insert(0, _p)

B, H, S, D = 2, 16, 2048, 64
N_CORES = 8
PAIRS = B * H                     # 32
PPC = PAIRS // N_CORES            # 4 pairs per core
SCALE = 1.0 / 8.0                 # 1/sqrt(64)

KT = S // 128                     # k-tiles of 128
QTILE = 512
QT = S // QTILE                   # q-tiles of 512
GROUP = 2                         # k-tiles per PSUM group (2 banks)
PP_BUFS = int(os.environ.get("PP_BUFS", "2"))
PD_BUFS = int(os.environ.get("PD_BUFS", "3"))
SC_BUFS = int(os.environ.get("SC_BUFS", "3"))
LAG = int(os.environ.get("LAG", "2"))
EPI_BUFS = int(os.environ.get("EPI_BUFS", "2"))

_CACHE = {}


def _pe_pattern(ngroups):
    """PE-path group mask: PE chunks at start/end of each qt, DVE between."""
    if ngroups >= 8:
        base = [True, False, False, True, False, False, False, True]
        reps = (ngroups + 7) // 8
        return (base * reps)[:ngroups]
    pat = [False] * ngroups
    pat[0] = True
    if ngroups > 2:
        pat[-1] = True
    return pat


def _build_nc():
    import concourse.mybir as mybir
    import concourse.tile as tile
    from concourse import bacc

    f32 = mybir.dt.float32
    f32r = mybir.dt.float32r
    nc = bacc.Bacc(None)

    bf16 = mybir.dt.bfloat16
    qT = nc.declare_dram_parameter("qT", [PPC, D, S], bf16, isOutput=False)
    kT = nc.declare_dram_parameter("kT", [PPC, D, S], bf16, isOutput=False)
    v1 = nc.declare_dram_parameter("v1", [PPC, S, D + 1], bf16, isOutput=False)
    biasT = nc.declare_dram_parameter("biasT", [S, S], bf16, isOutput=False)
    ident_d = nc.declare_dram_parameter("ident", [128, 128], bf16, isOutput=False)
    out = nc.declare_dram_parameter("out", [PPC, S, D], f32, isOutput=True)

    ngroups = KT // GROUP
    pe_path = _pe_pattern(ngroups)

    with tile.TileContext(nc) as tc:
        with (
            tc.tile_pool(name="const", bufs=1) as const_pool,
            tc.tile_pool(name="bias", bufs=1) as bias_pool,
            tc.tile_pool(name="qk", bufs=2) as qk_pool,
            tc.tile_pool(name="vv", bufs=2) as v_pool,
            tc.tile_pool(name="probP", bufs=PP_BUFS) as pP_pool,
            tc.tile_pool(name="probD", bufs=PD_BUFS) as pD_pool,
            tc.tile_pool(name="epi", bufs=EPI_BUFS) as epi_pool,
            tc.tile_pool(name="sc", bufs=SC_BUFS, space="PSUM") as sc_pool,
            tc.tile_pool(name="acc", bufs=1, space="PSUM") as acc_pool,
            tc.tile_pool(name="tp", bufs=1, space="PSUM") as tp_pool,
        ):
            bf16 = mybir.dt.bfloat16
            ident = const_pool.tile([128, 128], bf16)
            nc.sync.dma_start(ident[:], ident_d[:])
            # f32 copy for the (fp32-only) PE transpose path
            ident_f = const_pool.tile([128, 128], f32)
            nc.vector.tensor_scalar_mul(ident_f[:], ident[:], 1.0)

            def load_pair(p):
                qT_sb = qk_pool.tile([D, S], bf16, tag="q")
                nc.sync.dma_start(qT_sb[:], qT[p])
                # Q pre-scale by 1/8 (exact in fp32) -> no scale elsewhere.
                nc.vector.tensor_scalar_mul(qT_sb[:], qT_sb[:], SCALE)
                kT_sb = qk_pool.tile([D, S], bf16, tag="k")
                nc.sync.dma_start(kT_sb[:], kT[p])
                # V already has the ones-column appended (host-side), so the
                # second matmul also yields sum(exp) in row D.
                v_sb = v_pool.tile([128, KT, D + 1], bf16)
                nc.sync.dma_start(
                    v_sb[:], v1[p].rearrange("(kt p) d -> p kt d", p=128)
                )
                return qT_sb, kT_sb, v_sb

            # pair 0 loads first so MM1 can start before the bias stream.
            loaded = {0: load_pair(0)}

            # Full bias^T resident in SBUF: [128, KT, S] (128 KiB/partition).
            bias_sb = bias_pool.tile([128, KT, S], bf16)
            bias_src = biasT.rearrange("(kt p) q -> p kt q", p=128)
            # q-column-major chunks so qt=0's bias slice lands first
            for qc in range(QT):
                for kt in range(KT):
                    nc.sync.dma_start(
                        bias_sb[:, kt, qc * QTILE : (qc + 1) * QTILE],
                        bias_src[:, kt, qc * QTILE : (qc + 1) * QTILE],
                    )

            # ---- global chunk stream over (pair, qt, chunk) ----------------
            def make_chunk_plan():
                plans = []
                g = 0
                while g < ngroups:
                    if pe_path[g]:
                        plans.append([g])
                        g += 1
                    else:
                        gl = [g]
                        if g + 1 < ngroups and not pe_path[g + 1]:
                            gl.append(g + 1)
                        plans.append(gl)
                        g += len(gl)
                return plans

            chunk_plans = make_chunk_plan()

            stream = []  # (p, qt, g_list, is_last_of_qt)
            for p in range(PPC):
                for qt in range(QT):
                    for ci, gl in enumerate(chunk_plans):
                        stream.append((p, qt, gl, ci == len(chunk_plans) - 1))

            state = {}  # (p, qt) -> dict with o_psum, tiles

            def produce(p, qt, g_list):
                qT_sb, kT_sb, v_sb = loaded[p]
                qs = qT_sb[:, qt * QTILE : (qt + 1) * QTILE]
                kt0 = g_list[0] * GROUP
                if pe_path[g_list[0]]:
                    s_psum = sc_pool.tile([128, GROUP, QTILE], f32)
                    for j in range(GROUP):
                        nc.tensor.matmul(
                            s_psum[:, j, :],
                            kT_sb[:, (kt0 + j) * 128 : (kt0 + j + 1) * 128],
                            qs,
                            start=True,
                            stop=False,
                        )
                        nc.tensor.matmul(
                            s_psum[:, j, :],
                            ident[:],
                            bias_sb[:, kt0 + j, qt * QTILE : (qt + 1) * QTILE],
                            start=False,
                            stop=True,
                        )
                    p_sb = pP_pool.tile([128, GROUP, QTILE], bf16, tag="pP")
                    nc.scalar.activation(
                        p_sb[:], s_psum[:], mybir.ActivationFunctionType.Exp
                    )
                    return (kt0, GROUP, p_sb)
                nk = len(g_list) * GROUP
                p_sb = pD_pool.tile([128, 2 * GROUP, QTILE], bf16, tag="pD")
                for gg in range(len(g_list)):
                    s_psum = sc_pool.tile([128, GROUP, QTILE], f32)
                    for j in range(GROUP):
                        kt = kt0 + gg * GROUP + j
                        nc.tensor.matmul(
                            s_psum[:, j, :],
                            kT_sb[:, kt * 128 : (kt + 1) * 128],
                            qs,
                            start=True,
                            stop=True,
                        )
                    nc.vector.scalar_tensor_tensor(
                        p_sb[:, gg * GROUP : (gg + 1) * GROUP, :],
                        s_psum[:],
                        1.0,
                        bias_sb[
                            :,
                            kt0 + gg * GROUP : kt0 + (gg + 1) * GROUP,
                            qt * QTILE : (qt + 1) * QTILE,
                        ],
                        op0=mybir.AluOpType.mult,
                        op1=mybir.AluOpType.add,
                    )
                nc.scalar.activation(
                    p_sb[:, :nk, :],
                    p_sb[:, :nk, :],
                    mybir.ActivationFunctionType.Exp,
                )
                return (kt0, nk, p_sb)

            def consume(p, qt, chunk):
                _, _, v_sb = loaded[p]
                st = state[(p, qt)]
                kt0, nk, p_sb = chunk
                for j in range(nk):
                    kt = kt0 + j
                    nc.tensor.matmul(
                        st["o_psum"][:],
                        v_sb[:, kt, :],
                        p_sb[:, j, :],
                        start=(kt == 0),
                        stop=(kt == KT - 1),
                    )

            def epilogue(p, qt):
                st = state.pop((p, qt))
                o_psum = st["o_psum"]
                o_sb = epi_pool.tile([D + 1, QTILE], f32, tag="osb")
                nc.vector.tensor_scalar_mul(o_sb[:], o_psum[:], 1.0)
                for c in range(4):
                    t_psum = tp_pool.tile([128, D + 1], f32, tag="tp")
                    nc.tensor.transpose(
                        t_psum[:],
                        o_sb[:, c * 128 : (c + 1) * 128],
                        ident_f[: D + 1, : D + 1],
                    )
                    r_sb = epi_pool.tile([128, 1], f32, tag="rsb")
                    nc.vector.reciprocal(r_sb[:], t_psum[:, D : D + 1])
                    f_sb = epi_pool.tile([128, D], f32, tag="fsb")
                    nc.vector.tensor_scalar_mul(f_sb[:], t_psum[:, :D], r_sb[:])
                    row0 = qt * QTILE + c * 128
                    nc.sync.dma_start(out[p, row0 : row0 + 128, :], f_sb[:])

            pending = []  # (p, qt, chunk, is_last)
            pending_epi = []  # (p, qt) awaiting epilogue, with lag
            for i, (p, qt, gl, is_last) in enumerate(stream):
                if p not in loaded:
                    loaded[p] = load_pair(p)
                # prefetch next pair during this pair's last q-tile
                if qt == QT - 1 and p + 1 < PPC and p + 1 not in loaded:
                    loaded[p + 1] = load_pair(p + 1)
                # drop stale pair handles (pair p-1 is still consumed at
                # p's first chunk via the lag-1 pipeline)
                for old in [k for k in loaded if k < p - 1]:
                    del loaded[old]
                if (p, qt) not in state:
                    o_psum = acc_pool.tile(
                        [D + 1, QTILE], mybir.dt.float32, name="osum", tag="osum"
                    )
                    state[(p, qt)] = {"o_psum": o_psum}
                chunk = produce(p, qt, gl)
                if len(pending) >= LAG:
                    pp, pq, pc, plast = pending.pop(0)
                    consume(pp, pq, pc)
                    if plast:
                        epilogue(pp, pq)
                pending.append((p, qt, chunk, is_last))
            while pending:
                pp, pq, pc, plast = pending.pop(0)
                consume(pp, pq, pc)
                if plast:
                    epilogue(pp, pq)

    return nc


def _get_nc():
    if "nc" not in _CACHE:
        nc = _build_nc()
        nc.finalize()
        _CACHE["nc"] = nc
    return _CACHE["nc"]


def _make_in_maps(mat1, mat2, mat3, bias):
    import ml_dtypes

    bf16 = ml_dtypes.bfloat16
    q = np.ascontiguousarray(np.asarray(mat1, dtype=np.float32).reshape(PAIRS, S, D))
    k = np.ascontiguousarray(np.asarray(mat2, dtype=np.float32).reshape(PAIRS, S, D))
    v = np.asarray(mat3, dtype=np.float32).reshape(PAIRS, S, D)
    v1 = np.concatenate([v, np.ones((PAIRS, S, 1), np.float32)], axis=2)
    v1 = np.ascontiguousarray(v1)
    biasT = np.ascontiguousarray(np.asarray(bias, dtype=np.float32).reshape(S, S).T.astype(bf16))
    ident = np.eye(128, dtype=np.float32).astype(bf16)

    in_maps = []
    for c in range(N_CORES):
        sl = slice(c * PPC, (c + 1) * PPC)
        in_maps.append(
            {
                "qT": np.ascontiguousarray(q[sl].transpose(0, 2, 1).astype(bf16)),
                "kT": np.ascontiguousarray(k[sl].transpose(0, 2, 1).astype(bf16)),
                "v1": np.ascontiguousarray(v1[sl].astype(bf16)),
                "biasT": biasT,
                "ident": ident,
            }
        )
    return in_maps


def kernel(mat1, mat2, mat3, bias):
    from concourse.bass_utils import run_bass_kernel_spmd

    in_maps = _make_in_maps(mat1, mat2, mat3, bias)
    nc = _get_nc()
    _CACHE["in_maps"] = in_maps
    res = run_bass_kernel_spmd(nc, in_maps, list(range(N_CORES)))
    outs = [res.results[c]["out"] for c in range(N_CORES)]
    full = np.concatenate(outs, axis=0).reshape(B, H, S, D)
    return full.astype(np.float32)



# revision 4
# speedup vs baseline: 1.1212x; 1.1212x over previous
"""MHA forward kernel for Trainium2 (Bass/Tile), sharded over (batch, head)
pairs across 8 NeuronCores.

Math (per (b,h) pair):
    out = softmax(Q K^T / sqrt(64) + bias) @ V     # bias broadcast over (b,h)

Device-side decomposition (everything transposed so the S x S score matrix
never needs an on-chip transpose):
    scoresT[k, q] = sum_d K[k,d] Q'[q,d]          (Q' = Q/8, host pre-scaled)
    es = exp(scoresT)                              (ACT, PSUM -> SBUF bf16)
    p  = es * ebiasT                               (DVE, ebiasT = exp(bias)^T
                                                    host-precomputed bf16;
                                                    exp(s+b) = exp(s)exp(b))
    outT[d, q], sums[q] = [V | ones] matmul accum over k   (PE)
    host epilogue: out[q, d] = outT[d, q] / sums[q]

Engine budget per core (4 pairs, S=2048): PE does only the two matmul
streams (~109us at 2.4GHz); ACT does all exp (~133us, bottleneck); DVE
does the ebias multiplies (bf16 2x mode) + PSUM drains (~95us). The
final divide + transpose runs on host over the gathered f32 output.
"""

import os
import sys

import numpy as np

for _p in ("/opt/trn_rl_repo",):
    if _p not in sys.path and os.path.isdir(_p):
        sys.path.insert(0, _p)

B, H, S, D = 2, 16, 2048, 64
N_CORES = 8
PAIRS = B * H                     # 32
PPC = PAIRS // N_CORES            # 4 pairs per core
SCALE = 1.0 / 8.0                 # 1/sqrt(64)

KT = S // 128                     # 16 k-tiles of 128
QTILE = 512
QT = S // QTILE                   # 4 q-tiles
GROUP = 2                         # k-tiles per PSUM scores group (2 banks)
NG = KT // GROUP                  # 8 groups per q-tile
SC_BUFS = int(os.environ.get("SC_BUFS", "3"))
ES_BUFS = int(os.environ.get("ES_BUFS", "3"))
P_BUFS = int(os.environ.get("P_BUFS", "3"))
LAG = int(os.environ.get("LAG", "2"))

_CACHE = {}


def _build_nc():
    import concourse.mybir as mybir
    import concourse.tile as tile
    from concourse import bacc

    f32 = mybir.dt.float32
    bf16 = mybir.dt.bfloat16
    nc = bacc.Bacc(None)

    qT = nc.declare_dram_parameter("qT", [PPC, D, S], bf16, isOutput=False)
    kT = nc.declare_dram_parameter("kT", [PPC, D, S], bf16, isOutput=False)
    v1 = nc.declare_dram_parameter("v1", [PPC, S, D + 1], bf16, isOutput=False)
    ebT = nc.declare_dram_parameter("ebT", [S, S], bf16, isOutput=False)
    outT = nc.declare_dram_parameter("outT", [PPC, QT, D + 1, QTILE], f32, isOutput=True)

    with tile.TileContext(nc) as tc:
        with (
            tc.tile_pool(name="eb", bufs=1) as eb_pool,
            tc.tile_pool(name="qk", bufs=2) as qk_pool,
            tc.tile_pool(name="vv", bufs=2) as v_pool,
            tc.tile_pool(name="es", bufs=ES_BUFS) as es_pool,
            tc.tile_pool(name="pp", bufs=P_BUFS) as p_pool,
            tc.tile_pool(name="ob", bufs=2) as ob_pool,
            tc.tile_pool(name="sc", bufs=SC_BUFS, space="PSUM") as sc_pool,
            tc.tile_pool(name="acc", bufs=2, space="PSUM") as acc_pool,
        ):
            def load_pair(p):
                qT_sb = qk_pool.tile([D, S], bf16, tag="q")
                nc.sync.dma_start(qT_sb[:], qT[p])
                kT_sb = qk_pool.tile([D, S], bf16, tag="k")
                nc.sync.dma_start(kT_sb[:], kT[p])
                # V with ones-column appended (host-side): MM2 row D is sum(p).
                v_sb = v_pool.tile([128, KT, D + 1], bf16, tag="v")
                nc.sync.dma_start(
                    v_sb[:], v1[p].rearrange("(kt p) d -> p kt d", p=128)
                )
                return qT_sb, kT_sb, v_sb

            loaded = {0: load_pair(0)}

            # Full exp(bias)^T resident in SBUF: [128, KT, S] (64 KiB/part).
            # q-column-major so qt=0 chunks land first.
            eb_sb = eb_pool.tile([128, KT, S], bf16)
            eb_src = ebT.rearrange("(kt p) q -> p kt q", p=128)
            for qc in range(QT):
                for kt in range(KT):
                    nc.sync.dma_start(
                        eb_sb[:, kt, qc * QTILE : (qc + 1) * QTILE],
                        eb_src[:, kt, qc * QTILE : (qc + 1) * QTILE],
                    )

            # ---- chunk stream over (pair, qt, group) -----------------------
            stream = []  # (p, qt, g)
            for p in range(PPC):
                for qt in range(QT):
                    for g in range(NG):
                        stream.append((p, qt, g))

            state = {}

            def produce(p, qt, g):
                qT_sb, kT_sb, _ = loaded[p]
                qs = qT_sb[:, qt * QTILE : (qt + 1) * QTILE]
                s_psum = sc_pool.tile([128, GROUP, QTILE], f32, tag="sc")
                for j in range(GROUP):
                    kt = g * GROUP + j
                    nc.tensor.matmul(
                        s_psum[:, j, :],
                        kT_sb[:, kt * 128 : (kt + 1) * 128],
                        qs,
                        start=True,
                        stop=True,
                    )
                es = es_pool.tile([128, GROUP, QTILE], bf16, tag="es")
                nc.scalar.activation(
                    es[:], s_psum[:], mybir.ActivationFunctionType.Exp
                )
                p_sb = p_pool.tile([128, GROUP, QTILE], bf16, tag="p")
                nc.vector.tensor_mul(
                    p_sb[:],
                    es[:],
                    eb_sb[
                        :,
                        g * GROUP : (g + 1) * GROUP,
                        qt * QTILE : (qt + 1) * QTILE,
                    ],
                )
                return p_sb

            def consume(p, qt, g, p_sb):
                _, _, v_sb = loaded[p]
                st = state[(p, qt)]
                for j in range(GROUP):
                    kt = g * GROUP + j
                    nc.tensor.matmul(
                        st,
                        v_sb[:, kt, :],
                        p_sb[:, j, :],
                        start=(kt == 0),
                        stop=(kt == KT - 1),
                    )

            def epilogue(p, qt):
                o_psum = state.pop((p, qt))
                o_sb = ob_pool.tile([D + 1, QTILE], f32, tag="osb")
                nc.vector.tensor_scalar_mul(o_sb[:], o_psum[:], 1.0)
                nc.sync.dma_start(outT[p, qt], o_sb[:])

            pending = []  # (p, qt, g, p_sb)
            for i, (p, qt, g) in enumerate(stream):
                # prefetch next pair's tensors early (DMA has slack)
                if qt == 0 and g == 0 and p + 1 < PPC and p + 1 not in loaded:
                    loaded[p + 1] = load_pair(p + 1)
                if (p, qt) not in state:
                    state[(p, qt)] = acc_pool.tile(
                        [D + 1, QTILE], mybir.dt.float32, name="osum", tag="osum"
                    )
                p_sb = produce(p, qt, g)
                if len(pending) >= LAG:
                    pp, pq, pg, ps = pending.pop(0)
                    consume(pp, pq, pg, ps)
                    if pg == NG - 1:
                        epilogue(pp, pq)
                pending.append((p, qt, g, p_sb))
            while pending:
                pp, pq, pg, ps = pending.pop(0)
                consume(pp, pq, pg, ps)
                if pg == NG - 1:
                    epilogue(pp, pq)

    return nc


def _get_nc():
    if "nc" not in _CACHE:
        nc = _build_nc()
        nc.finalize()
        _CACHE["nc"] = nc
    return _CACHE["nc"]


def _make_in_maps(mat1, mat2, mat3, bias):
    import ml_dtypes

    bf16 = ml_dtypes.bfloat16
    q = np.asarray(mat1, dtype=np.float32).reshape(PAIRS, S, D) * np.float32(SCALE)
    k = np.asarray(mat2, dtype=np.float32).reshape(PAIRS, S, D)
    v = np.asarray(mat3, dtype=np.float32).reshape(PAIRS, S, D)
    v1 = np.concatenate([v, np.ones((PAIRS, S, 1), np.float32)], axis=2)
    ebT = np.ascontiguousarray(
        np.exp(np.asarray(bias, dtype=np.float32).reshape(S, S)).T.astype(bf16)
    )

    in_maps = []
    for c in range(N_CORES):
        sl = slice(c * PPC, (c + 1) * PPC)
        in_maps.append(
            {
                "qT": np.ascontiguousarray(q[sl].transpose(0, 2, 1).astype(bf16)),
                "kT": np.ascontiguousarray(k[sl].transpose(0, 2, 1).astype(bf16)),
                "v1": np.ascontiguousarray(v1[sl].astype(bf16)),
                "ebT": ebT,
            }
        )
    return in_maps


def kernel(mat1, mat2, mat3, bias):
    from concourse.bass_utils import run_bass_kernel_spmd

    in_maps = _make_in_maps(mat1, mat2, mat3, bias)
    nc = _get_nc()
    _CACHE["in_maps"] = in_maps
    res = run_bass_kernel_spmd(nc, in_maps, list(range(N_CORES)))
    outs = []
    for c in range(N_CORES):
        oT = res.results[c]["outT"]            # [PPC, QT, D+1, QTILE] f32
        oT = oT.transpose(0, 2, 1, 3).reshape(PPC, D + 1, S)
        o = oT[:, :D, :] / oT[:, D : D + 1, :]  # divide by softmax sums
        outs.append(o.transpose(0, 2, 1))       # [PPC, S, D]
    full = np.concatenate(outs, axis=0).reshape(B, H, S, D)
    return np.ascontiguousarray(full.astype(np.float32))
